# revision 16
# baseline (speedup 1.0000x reference)
"""Distributed Trainium2 kernel for nn_ActionEmbeddingModel.

Reference computation (B=4096, DC=1024, A=20000, C=128, H=1024):
    h         = relu(context @ w1 + b1)          # [B, H]
    ctx_score = h @ w2[:H]                       # [B]
    act_score = emb @ w2[H:]                     # [A]
    out[b, a] = ctx_score[b] + act_score[a] + b2 # [B, A]

Sharding (8 cores): data-parallel over the batch for context/h/ctx_score.
emb is sharded over actions for an AllGather of act_score; additionally
each core receives its own + three neighboring emb shards and computes those
act rows locally, so the first three output chunks depend only on local
work — the AllGather latency (collective entry + cross-core launch skew)
hides behind them. Device chunk j holds global action block (i+j) % 8;
the host un-rotates the column blocks when assembling the full output.

The [B/8, A] output shard is generated PE-free: act rows are partition-
broadcast on GpSimd and per-batch-row scores are added as per-partition
scalars on DVE/ACT, so the output phase is purely DMA-bound (its floor is
the 41 MB/core output write at ~358 GB/s HBM per-core bandwidth).

Matmuls run in float32r (fp32 bits, single-pass PE streaming, ~1.5e-4
rel err). Host-side prep only reorders memory; all FLOPs run on device.
"""

import numpy as np

import concourse.bass as bass
import concourse.mybir as mybir
from concourse import bacc
import concourse.tile as tile
from concourse.tile import TileContext
from concourse.bass_utils import run_bass_kernel_spmd

# Problem shape (hardcoded per harness contract).
B, DC, A, C, H = 4096, 1024, 20000, 128, 1024
N_CORES = 8
B_SH = B // N_CORES        # 512 batch rows per core
A_SH = A // N_CORES        # 2500 actions per block / emb shard
P = 128                    # partitions
KT = DC // P               # 8 contraction tiles for fc1
HT = H // P                # 8 hidden tiles
BT = B_SH // P             # 4 batch chunks of 128 rows
MM_N = 500                 # matmul free-dim chunk (<=512, even for f32r)
N_LOC = 4                  # act blocks computed locally (own + 3 neighbors)
F32 = mybir.dt.float32
F32R = mybir.dt.float32r

_CACHED_NC = None


def _build():
    nc = bacc.Bacc(num_devices=N_CORES)

    ctx_pp = nc.declare_dram_parameter("ctx_pp", [P, KT, B_SH], F32R, isOutput=False)
    w1_pp = nc.declare_dram_parameter("w1_pp", [HT, P, KT, P], F32R, isOutput=False)
    b1c = nc.declare_dram_parameter("b1c", [P, HT], F32, isOutput=False)
    w2h = nc.declare_dram_parameter("w2h", [P, HT], F32R, isOutput=False)
    w2c = nc.declare_dram_parameter("w2c", [P, 1], F32R, isOutput=False)
    b2 = nc.declare_dram_parameter("b2", [1, 1], F32, isOutput=False)
    one1 = nc.declare_dram_parameter("one1", [1, 1], F32, isOutput=False)
    # Per-core emb shards: index 0 = own shard (rank i), 1/2 = ranks i+1, i+2.
    embs = [
        nc.declare_dram_parameter(f"embT{j}", [C, A_SH], F32R, isOutput=False)
        for j in range(N_LOC)
    ]
    out_ext = nc.declare_dram_parameter("out", [B_SH, A], F32, isOutput=True)

    # Collective bounce buffers (collectives can't touch I/O tensors).
    ag_in = nc.dram_tensor("ag_in", [A_SH], F32)
    ag_out = nc.dram_tensor("ag_out", [A], F32, addr_space="Shared")

    relu = mybir.ActivationFunctionType.Relu
    ident = mybir.ActivationFunctionType.Identity

    with TileContext(nc, num_cores=N_CORES) as tc:
        with (
            tc.tile_pool(name="persist", bufs=1) as persist,
            tc.tile_pool(name="psum_h", bufs=4, space="PSUM") as pp,
            tc.tile_pool(name="psum_v", bufs=2, space="PSUM") as pp1,
            tc.tile_pool(name="psum_tr", bufs=1, space="PSUM") as trp,
        ):
            ctx_col = persist.tile([P, BT], F32, tag="ctx_col")

            # ---- input DMAs (sync HWDGE ring; own emb shard first, the
            # ---- neighbor shards are deferred until after ctx/w1) ----
            emb_sbs = []
            for j in range(N_LOC):
                e = persist.tile([C, A_SH], F32R, tag=f"emb{j}")
                if j == 0:
                    nc.sync.dma_start(out=e[:, :], in_=embs[j][:, :])
                emb_sbs.append(e)
            w2c_sb = persist.tile([P, 1], F32R, tag="w2c")
            nc.sync.dma_start(out=w2c_sb[:, :], in_=w2c[:, :])
            b2_sb = persist.tile([1, 1], F32, tag="b2")
            nc.sync.dma_start(out=b2_sb[:, :], in_=b2[:, :])
            b1_sb = persist.tile([P, HT], F32, tag="b1")
            nc.sync.dma_start(out=b1_sb[:, :], in_=b1c[:, :])
            w2h_sb = persist.tile([P, HT], F32R, tag="w2h")
            nc.sync.dma_start(out=w2h_sb[:, :], in_=w2h[:, :])
            one_sb = persist.tile([1, 1], F32, tag="one1")
            nc.sync.dma_start(out=one_sb[:, :], in_=one1[:, :])

            # ---- local act rows (f32r matvecs); row 0 feeds the AllGather ----
            # GpSimd library warm-up so the first real bcast is hot.
            warm = persist.tile([P, 8], F32, tag="warm")
            nc.gpsimd.partition_broadcast(warm[:, :], b1_sb[0:1, 0:8])
            act_rows = []
            last_mm = None
            for j in range(1):
                ar = persist.tile([1, A_SH], F32, tag=f"act{j}")
                for at in range(A_SH // MM_N):
                    ps = pp1.tile([1, MM_N], F32, tag="act_ps")
                    last_mm = nc.tensor.matmul(
                        ps[:, :],
                        w2c_sb[:, :],
                        emb_sbs[j][:, at * MM_N:(at + 1) * MM_N],
                        start=True,
                        stop=True,
                    )
                    nc.scalar.add(
                        ar[:, at * MM_N:(at + 1) * MM_N],
                        ps[:, :],
                        b2_sb[0:1, 0:1],
                    )
                act_rows.append(ar)
            nc.gpsimd.dma_start(out=ag_in[None, :], in_=act_rows[0][0:1, :])
            nc.gpsimd.collective_compute(
                "AllGather",
                mybir.AluOpType.bypass,
                replica_groups=[list(range(N_CORES))],
                ins=[ag_in[:]],
                outs=[ag_out[:]],
            )

            # ---- hT = relu(w1.T @ ctx.T + b1) and ctx_col; w1/ctx in a
            # ---- scoped pool released before the output tiles need SBUF ----
            with tc.tile_pool(name="fc1_pool", bufs=1) as fc1p:
                ctx_sb = fc1p.tile([P, KT * B_SH], F32R, tag="ctx")
                nc.sync.dma_start(
                    out=ctx_sb[:, :].rearrange("p (kt n) -> p kt n", kt=KT),
                    in_=ctx_pp[:, :, :],
                )
                w1_sb = fc1p.tile([P, HT * KT * P], F32R, tag="w1")
                for hb in range(HT):
                    nc.sync.dma_start(
                        out=w1_sb[
                            :, hb * KT * P:(hb + 1) * KT * P
                        ].rearrange("p (kt c) -> p kt c", kt=KT),
                        in_=w1_pp[hb, :, :, :],
                    )

                ht_tiles = []
                for ht in range(HT):
                    ps = pp.tile([P, B_SH], F32, tag="h_ps")
                    for kt in range(KT):
                        base = ht * KT * P + kt * P
                        mm = nc.tensor.matmul(
                            ps[:, :],
                            w1_sb[:, base:base + P],
                            ctx_sb[:, kt * B_SH:(kt + 1) * B_SH],
                            start=(kt == 0),
                            stop=(kt == KT - 1),
                        )
                        if ht == 0 and kt == 0:
                            tile.add_dep_helper(
                                mm.ins, last_mm.ins, sync=False,
                                reason="PE: local act matvecs first",
                            )
                    hts = fc1p.tile([P, B_SH], F32R, tag=f"ht{ht}")
                    nc.scalar.activation(
                        hts[:, :], ps[:, :], relu, bias=b1_sb[:, ht:ht + 1]
                    )
                    ht_tiles.append(hts)

                # ---- ctx_score row then transpose to ctx_col [128, BT] ----
                psc = pp1.tile([1, B_SH], F32, tag="act_ps")
                for ht in range(HT):
                    nc.tensor.matmul(
                        psc[:, :],
                        w2h_sb[:, ht:ht + 1],
                        ht_tiles[ht][:, :],
                        start=(ht == 0),
                        stop=(ht == HT - 1),
                    )
                ctx_row = persist.tile([1, B_SH], F32, tag="ctx_row")
                ctx_row_cp = nc.vector.tensor_copy(ctx_row[:, :], psc[:, :])
                last_tr_cp = None
                for bs in range(BT):
                    pst = trp.tile([P, 1], F32, tag="tr_ps")
                    nc.tensor.matmul(
                        pst[:, :],
                        ctx_row[0:1, bs * P:(bs + 1) * P],
                        one_sb[0:1, 0:1],
                        start=True,
                        stop=True,
                    )
                    last_tr_cp = nc.scalar.copy(ctx_col[:, bs:bs + 1], pst[:, :])

                # ---- neighbor act rows (needed from ~chunk-1 time on) ----
                for j in range(1, N_LOC):
                    nc.sync.dma_start(out=emb_sbs[j][:, :], in_=embs[j][:, :])
                    ar = persist.tile([1, A_SH], F32, tag=f"act{j}")
                    for at in range(A_SH // MM_N):
                        ps = pp1.tile([1, MM_N], F32, tag="act_ps")
                        mm = nc.tensor.matmul(
                            ps[:, :],
                            w2c_sb[:, :],
                            emb_sbs[j][:, at * MM_N:(at + 1) * MM_N],
                            start=True,
                            stop=True,
                        )
                        nc.scalar.add(
                            ar[:, at * MM_N:(at + 1) * MM_N],
                            ps[:, :],
                            b2_sb[0:1, 0:1],
                        )
                    act_rows.append(ar)

            # ---- output: device chunk j = global action block (pid+j)%8.
            # ---- Chunks 0..N_LOC-1 use local act rows; the rest read the
            # ---- AllGather at a dynamic (partition-id dependent) offset ----
            pid = nc.partition_id()
            with (
                tc.tile_pool(name="outp", bufs=5) as outp,
                tc.tile_pool(name="abcp", bufs=3) as abcp,
                tc.tile_pool(name="arowp", bufs=2) as arowp,
            ):
                for j in range(N_CORES):
                    if j < N_LOC:
                        src_row = act_rows[j]
                    else:
                        src_row = arowp.tile([1, A_SH], F32, tag="arow")
                        off = ((pid + j) % N_CORES) * A_SH
                        nc.gpsimd.dma_start(
                            out=src_row[:, :],
                            in_=ag_out[None, bass.ds(off, A_SH)],
                        )
                    act_bc = abcp.tile([P, A_SH], F32, tag="abc")
                    nc.gpsimd.partition_broadcast(act_bc[:, :], src_row[0:1, :])
                    for bs in range(BT):
                        o_sb = outp.tile([P, A_SH], F32, tag="osb")
                        if (j * BT + bs) % 2:
                            add_o = nc.scalar.activation(
                                o_sb[:, :], act_bc[:, :], ident,
                                bias=ctx_col[:, bs:bs + 1],
                            )
                            if j == 0 and bs == 1:
                                tile.add_dep_helper(
                                    add_o.ins, last_tr_cp.ins, sync=False,
                                    reason="ACT: out adds after tr copies",
                                )
                        else:
                            add_o = nc.vector.tensor_scalar_add(
                                o_sb[:, :], act_bc[:, :], ctx_col[:, bs:bs + 1]
                            )
                            if j == 0 and bs == 0:
                                tile.add_dep_helper(
                                    add_o.ins, ctx_row_cp.ins, sync=False,
                                    reason="DVE: out adds after ctx_row copy",
                                )
                        nc.sync.dma_start(
                            out=out_ext[
                                bs * P:(bs + 1) * P, j * A_SH:(j + 1) * A_SH
                            ],
                            in_=o_sb[:, :],
                        )
    nc.finalize()
    return nc


def _get_nc():
    global _CACHED_NC
    if _CACHED_NC is None:
        _CACHED_NC = _build()
    return _CACHED_NC


def _in_maps(context, w1, b1, emb, w2, b2):
    context = np.asarray(context, dtype=np.float32)
    w1 = np.asarray(w1, dtype=np.float32)
    b1 = np.asarray(b1, dtype=np.float32)
    emb = np.asarray(emb, dtype=np.float32)
    w2 = np.asarray(w2, dtype=np.float32)
    b2 = np.asarray(b2, dtype=np.float32)

    # w1_pp[hb, p, kt, c] = w1[kt*P + p, hb*P + c]
    w1_pp = np.ascontiguousarray(
        w1.reshape(KT, P, HT, P).transpose(2, 1, 0, 3)
    )
    b1c = np.ascontiguousarray(b1.reshape(HT, P).T)
    w2h = np.ascontiguousarray(w2[:H].reshape(HT, P).T)
    w2c = np.ascontiguousarray(w2[H:].reshape(P, 1))
    b2m = b2.reshape(1, 1)
    one1 = np.ones((1, 1), dtype=np.float32)
    emb_sh = [
        np.ascontiguousarray(emb[r * A_SH:(r + 1) * A_SH].T)
        for r in range(N_CORES)
    ]

    maps = []
    for i in range(N_CORES):
        ctx_sh = context[i * B_SH:(i + 1) * B_SH]
        # ctx_pp[p, kt, n] = context[n, kt*P + p]
        ctx_pp = np.ascontiguousarray(
            ctx_sh.T.reshape(KT, P, B_SH).transpose(1, 0, 2)
        )
        m = {
            "ctx_pp": ctx_pp,
            "w1_pp": w1_pp,
            "b1c": b1c,
            "w2h": w2h,
            "w2c": w2c,
            "b2": b2m,
            "one1": one1,
        }
        for j in range(N_LOC):
            m[f"embT{j}"] = emb_sh[(i + j) % N_CORES]
        maps.append(m)
    return maps


def kernel(context, w1, b1, emb, w2, b2, _trace=False, **_trace_kwargs):
    nc = _get_nc()
    maps = _in_maps(context, w1, b1, emb, w2, b2)
    res = run_bass_kernel_spmd(
        nc, maps, core_ids=list(range(N_CORES)), trace=_trace, **_trace_kwargs
    )
    out = np.empty((B, A), dtype=np.float32)
    for i in range(N_CORES):
        dev = res.results[i]["out"]
        for j in range(N_CORES):
            blk = (i + j) % N_CORES
            out[
                i * B_SH:(i + 1) * B_SH, blk * A_SH:(blk + 1) * A_SH
            ] = dev[:, j * A_SH:(j + 1) * A_SH]
    if _trace:
        return out, res
    return out


# revision 17
# speedup vs baseline: 1.0750x; 1.0750x over previous
"""Distributed Trainium2 kernel for nn_ActionEmbeddingModel.

Reference computation (B=4096, DC=1024, A=20000, C=128, H=1024):
    h         = relu(context @ w1 + b1)          # [B, H]
    ctx_score = h @ w2[:H]                       # [B]
    act_score = emb @ w2[H:]                     # [A]
    out[b, a] = ctx_score[b] + act_score[a] + b2 # [B, A]

Sharding (8 cores): data-parallel over the batch for context/h/ctx_score.
emb is sharded over actions for an AllGather of act_score; additionally
each core receives its own + three neighboring emb shards and computes those
act rows locally, so the first three output chunks depend only on local
work — the AllGather latency (collective entry + cross-core launch skew)
hides behind them. Device chunk j holds global action block (i+j) % 8;
the host un-rotates the column blocks when assembling the full output.

The [B/8, A] output shard is generated PE-free: act rows are partition-
broadcast on GpSimd and per-batch-row scores are added as per-partition
scalars on DVE/ACT, so the output phase is purely DMA-bound (its floor is
the 41 MB/core output write at ~358 GB/s HBM per-core bandwidth).

Matmuls run in float32r (fp32 bits, single-pass PE streaming, ~1.5e-4
rel err). Host-side prep only reorders memory; all FLOPs run on device.
"""

import numpy as np

import concourse.bass as bass
import concourse.mybir as mybir
from concourse import bacc
import concourse.tile as tile
from concourse.tile import TileContext
from concourse.bass_utils import run_bass_kernel_spmd

# Problem shape (hardcoded per harness contract).
B, DC, A, C, H = 4096, 1024, 20000, 128, 1024
N_CORES = 8
B_SH = B // N_CORES        # 512 batch rows per core
A_SH = A // N_CORES        # 2500 actions per block / emb shard
P = 128                    # partitions
KT = DC // P               # 8 contraction tiles for fc1
HT = H // P                # 8 hidden tiles
BT = B_SH // P             # 4 batch chunks of 128 rows
MM_N = 500                 # matmul free-dim chunk (<=512, even for f32r)
N_LOC = 4                  # act blocks computed locally (own + 3 neighbors)
F32 = mybir.dt.float32
F32R = mybir.dt.float32r

_CACHED_NC = None


def _build():
    nc = bacc.Bacc(num_devices=N_CORES)

    ctx_pp = nc.declare_dram_parameter("ctx_pp", [P, KT, B_SH], F32R, isOutput=False)
    w1_pp = nc.declare_dram_parameter("w1_pp", [HT, P, KT, P], F32R, isOutput=False)
    b1c = nc.declare_dram_parameter("b1c", [P, HT], F32, isOutput=False)
    w2h = nc.declare_dram_parameter("w2h", [P, HT], F32R, isOutput=False)
    w2c = nc.declare_dram_parameter("w2c", [P, 1], F32R, isOutput=False)
    b2 = nc.declare_dram_parameter("b2", [1, 1], F32, isOutput=False)
    one1 = nc.declare_dram_parameter("one1", [1, 1], F32, isOutput=False)
    # Per-core emb shards: index 0 = own shard (rank i), 1/2 = ranks i+1, i+2.
    embs = [
        nc.declare_dram_parameter(f"embT{j}", [C, A_SH], F32R, isOutput=False)
        for j in range(N_LOC)
    ]
    out_ext = nc.declare_dram_parameter("out", [B_SH, A], F32, isOutput=True)

    # Collective bounce buffers (collectives can't touch I/O tensors).
    ag_in = nc.dram_tensor("ag_in", [A_SH], F32)
    ag_out = nc.dram_tensor("ag_out", [A], F32, addr_space="Shared")

    relu = mybir.ActivationFunctionType.Relu
    ident = mybir.ActivationFunctionType.Identity

    with TileContext(nc, num_cores=N_CORES) as tc:
        with (
            tc.tile_pool(name="persist", bufs=1) as persist,
            tc.tile_pool(name="psum_h", bufs=4, space="PSUM") as pp,
            tc.tile_pool(name="psum_v", bufs=2, space="PSUM") as pp1,
            tc.tile_pool(name="psum_tr", bufs=1, space="PSUM") as trp,
        ):
            ctx_col = persist.tile([P, BT], F32, tag="ctx_col")

            # ---- small input DMAs first (b1 gates the relus) ----
            emb_sbs = []
            for j in range(N_LOC):
                e = persist.tile([C, A_SH], F32R, tag=f"emb{j}")
                emb_sbs.append(e)
            w2c_sb = persist.tile([P, 1], F32R, tag="w2c")
            nc.sync.dma_start(out=w2c_sb[:, :], in_=w2c[:, :])
            b2_sb = persist.tile([1, 1], F32, tag="b2")
            nc.sync.dma_start(out=b2_sb[:, :], in_=b2[:, :])
            b1_sb = persist.tile([P, HT], F32, tag="b1")
            nc.sync.dma_start(out=b1_sb[:, :], in_=b1c[:, :])
            w2h_sb = persist.tile([P, HT], F32R, tag="w2h")
            nc.sync.dma_start(out=w2h_sb[:, :], in_=w2h[:, :])
            one_sb = persist.tile([1, 1], F32, tag="one1")
            nc.sync.dma_start(out=one_sb[:, :], in_=one1[:, :])

            # ---- hT = relu(w1.T @ ctx.T + b1) and ctx_col; w1/ctx in a
            # ---- scoped pool released before the output tiles need SBUF ----
            with tc.tile_pool(name="fc1_pool", bufs=1) as fc1p:
                ctx_sb = fc1p.tile([P, KT * B_SH], F32R, tag="ctx")
                nc.sync.dma_start(
                    out=ctx_sb[:, :].rearrange("p (kt n) -> p kt n", kt=KT),
                    in_=ctx_pp[:, :, :],
                )
                w1_sb = fc1p.tile([P, HT * KT * P], F32R, tag="w1")
                for hb in range(HT):
                    nc.sync.dma_start(
                        out=w1_sb[
                            :, hb * KT * P:(hb + 1) * KT * P
                        ].rearrange("p (kt c) -> p kt c", kt=KT),
                        in_=w1_pp[hb, :, :, :],
                    )

                nc.sync.dma_start(out=emb_sbs[0][:, :], in_=embs[0][:, :])

                ht_tiles = []
                for ht in range(HT):
                    ps = pp.tile([P, B_SH], F32, tag="h_ps")
                    for kt in range(KT):
                        base = ht * KT * P + kt * P
                        mm = nc.tensor.matmul(
                            ps[:, :],
                            w1_sb[:, base:base + P],
                            ctx_sb[:, kt * B_SH:(kt + 1) * B_SH],
                            start=(kt == 0),
                            stop=(kt == KT - 1),
                        )
                        last_ht_mm = mm
                    hts = fc1p.tile([P, B_SH], F32R, tag=f"ht{ht}")
                    nc.scalar.activation(
                        hts[:, :], ps[:, :], relu, bias=b1_sb[:, ht:ht + 1]
                    )
                    ht_tiles.append(hts)

            # ---- local act rows (f32r matvecs); row 0 feeds the AllGather ----
                # GpSimd library warm-up so the first real bcast is hot.
                warm = persist.tile([P, 8], F32, tag="warm")
                nc.gpsimd.partition_broadcast(warm[:, :], b1_sb[0:1, 0:8])
                act_rows = []
                last_mm = None
                for j in range(1):
                    ar = persist.tile([1, A_SH], F32, tag=f"act{j}")
                    for at in range(A_SH // MM_N):
                        ps = pp1.tile([1, MM_N], F32, tag="act_ps")
                        last_mm = nc.tensor.matmul(
                            ps[:, :],
                            w2c_sb[:, :],
                            emb_sbs[j][:, at * MM_N:(at + 1) * MM_N],
                            start=True,
                            stop=True,
                        )
                        nc.scalar.add(
                            ar[:, at * MM_N:(at + 1) * MM_N],
                            ps[:, :],
                            b2_sb[0:1, 0:1],
                        )
                    act_rows.append(ar)
                nc.gpsimd.dma_start(out=ag_in[None, :], in_=act_rows[0][0:1, :])
                nc.gpsimd.collective_compute(
                    "AllGather",
                    mybir.AluOpType.bypass,
                    replica_groups=[list(range(N_CORES))],
                    ins=[ag_in[:]],
                    outs=[ag_out[:]],
                )


                # ---- ctx_score row then transpose to ctx_col [128, BT] ----
                psc = pp1.tile([1, B_SH], F32, tag="act_ps")
                for ht in range(HT):
                    mm = nc.tensor.matmul(
                        psc[:, :],
                        w2h_sb[:, ht:ht + 1],
                        ht_tiles[ht][:, :],
                        start=(ht == 0),
                        stop=(ht == HT - 1),
                    )
                    if ht == 0:
                        tile.add_dep_helper(
                            mm.ins, last_mm.ins, sync=False,
                            reason="PE: ctx matvec after act0",
                        )
                ctx_row = persist.tile([1, B_SH], F32, tag="ctx_row")
                ctx_row_cp = nc.vector.tensor_copy(ctx_row[:, :], psc[:, :])
                last_tr_cp = None
                for bs in range(BT):
                    pst = trp.tile([P, 1], F32, tag="tr_ps")
                    nc.tensor.matmul(
                        pst[:, :],
                        ctx_row[0:1, bs * P:(bs + 1) * P],
                        one_sb[0:1, 0:1],
                        start=True,
                        stop=True,
                    )
                    last_tr_cp = nc.scalar.copy(ctx_col[:, bs:bs + 1], pst[:, :])

                # ---- neighbor act rows (needed from ~chunk-1 time on) ----
                for j in range(1, N_LOC):
                    nc.sync.dma_start(out=emb_sbs[j][:, :], in_=embs[j][:, :])
                    ar = persist.tile([1, A_SH], F32, tag=f"act{j}")
                    for at in range(A_SH // MM_N):
                        ps = pp1.tile([1, MM_N], F32, tag="act_ps")
                        mm = nc.tensor.matmul(
                            ps[:, :],
                            w2c_sb[:, :],
                            emb_sbs[j][:, at * MM_N:(at + 1) * MM_N],
                            start=True,
                            stop=True,
                        )
                        nc.scalar.add(
                            ar[:, at * MM_N:(at + 1) * MM_N],
                            ps[:, :],
                            b2_sb[0:1, 0:1],
                        )
                    act_rows.append(ar)

            # ---- output: device chunk j = global action block (pid+j)%8.
            # ---- Chunks 0..N_LOC-1 use local act rows; the rest read the
            # ---- AllGather at a dynamic (partition-id dependent) offset ----
            pid = nc.partition_id()
            with (
                tc.tile_pool(name="outp", bufs=5) as outp,
                tc.tile_pool(name="abcp", bufs=3) as abcp,
                tc.tile_pool(name="arowp", bufs=2) as arowp,
            ):
                for j in range(N_CORES):
                    if j < N_LOC:
                        src_row = act_rows[j]
                    else:
                        src_row = arowp.tile([1, A_SH], F32, tag="arow")
                        off = ((pid + j) % N_CORES) * A_SH
                        nc.gpsimd.dma_start(
                            out=src_row[:, :],
                            in_=ag_out[None, bass.ds(off, A_SH)],
                        )
                    act_bc = abcp.tile([P, A_SH], F32, tag="abc")
                    nc.gpsimd.partition_broadcast(act_bc[:, :], src_row[0:1, :])
                    for bs in range(BT):
                        o_sb = outp.tile([P, A_SH], F32, tag="osb")
                        if (j * BT + bs) % 2:
                            add_o = nc.scalar.activation(
                                o_sb[:, :], act_bc[:, :], ident,
                                bias=ctx_col[:, bs:bs + 1],
                            )
                            if j == 0 and bs == 1:
                                tile.add_dep_helper(
                                    add_o.ins, last_tr_cp.ins, sync=False,
                                    reason="ACT: out adds after tr copies",
                                )
                        else:
                            add_o = nc.vector.tensor_scalar_add(
                                o_sb[:, :], act_bc[:, :], ctx_col[:, bs:bs + 1]
                            )
                            if j == 0 and bs == 0:
                                tile.add_dep_helper(
                                    add_o.ins, ctx_row_cp.ins, sync=False,
                                    reason="DVE: out adds after ctx_row copy",
                                )
                        nc.sync.dma_start(
                            out=out_ext[
                                bs * P:(bs + 1) * P, j * A_SH:(j + 1) * A_SH
                            ],
                            in_=o_sb[:, :],
                        )
    nc.finalize()
    return nc


def _get_nc():
    global _CACHED_NC
    if _CACHED_NC is None:
        _CACHED_NC = _build()
    return _CACHED_NC


def _in_maps(context, w1, b1, emb, w2, b2):
    context = np.asarray(context, dtype=np.float32)
    w1 = np.asarray(w1, dtype=np.float32)
    b1 = np.asarray(b1, dtype=np.float32)
    emb = np.asarray(emb, dtype=np.float32)
    w2 = np.asarray(w2, dtype=np.float32)
    b2 = np.asarray(b2, dtype=np.float32)

    # w1_pp[hb, p, kt, c] = w1[kt*P + p, hb*P + c]
    w1_pp = np.ascontiguousarray(
        w1.reshape(KT, P, HT, P).transpose(2, 1, 0, 3)
    )
    b1c = np.ascontiguousarray(b1.reshape(HT, P).T)
    w2h = np.ascontiguousarray(w2[:H].reshape(HT, P).T)
    w2c = np.ascontiguousarray(w2[H:].reshape(P, 1))
    b2m = b2.reshape(1, 1)
    one1 = np.ones((1, 1), dtype=np.float32)
    emb_sh = [
        np.ascontiguousarray(emb[r * A_SH:(r + 1) * A_SH].T)
        for r in range(N_CORES)
    ]

    maps = []
    for i in range(N_CORES):
        ctx_sh = context[i * B_SH:(i + 1) * B_SH]
        # ctx_pp[p, kt, n] = context[n, kt*P + p]
        ctx_pp = np.ascontiguousarray(
            ctx_sh.T.reshape(KT, P, B_SH).transpose(1, 0, 2)
        )
        m = {
            "ctx_pp": ctx_pp,
            "w1_pp": w1_pp,
            "b1c": b1c,
            "w2h": w2h,
            "w2c": w2c,
            "b2": b2m,
            "one1": one1,
        }
        for j in range(N_LOC):
            m[f"embT{j}"] = emb_sh[(i + j) % N_CORES]
        maps.append(m)
    return maps


def kernel(context, w1, b1, emb, w2, b2, _trace=False, **_trace_kwargs):
    nc = _get_nc()
    maps = _in_maps(context, w1, b1, emb, w2, b2)
    res = run_bass_kernel_spmd(
        nc, maps, core_ids=list(range(N_CORES)), trace=_trace, **_trace_kwargs
    )
    out = np.empty((B, A), dtype=np.float32)
    for i in range(N_CORES):
        dev = res.results[i]["out"]
        for j in range(N_CORES):
            blk = (i + j) % N_CORES
            out[
                i * B_SH:(i + 1) * B_SH, blk * A_SH:(blk + 1) * A_SH
            ] = dev[:, j * A_SH:(j + 1) * A_SH]
    if _trace:
        return out, res
    return out
